# revision 1
# baseline (speedup 1.0000x reference)
"""Trainium2 Bass kernel for causal GQA self-attention (B=2,S=2048,D=1024,H=16,HKV=4,HD=64).

Sharding: 8 cores = DP(2 over batch) x TP(4 over GQA groups).
Each core computes, for one batch element and one GQA group (4 q heads + 1 kv head),
the partial output  y_group @ Wo[:, group_cols].T  (row-sharded Wo).
Host sums the 4 TP partials per batch element.
"""

import sys
from contextlib import ExitStack

sys.path.insert(0, "/opt/trn_rl_repo")

import numpy as np
import ml_dtypes

import concourse.bass as bass
import concourse.bacc as bacc
import concourse.tile as tile
import concourse.mybir as mybir
from concourse.bass_utils import run_bass_kernel_spmd

BF16 = mybir.dt.bfloat16
F32 = mybir.dt.float32
AF = mybir.ActivationFunctionType
BF16NP = ml_dtypes.bfloat16

D, H, HKV, HD, B, S = 1024, 16, 4, 64, 2, 2048
HG = 4              # q heads per core
KV_DIM = HKV * HD   # 256
E = HG * HD         # 256 local q-proj dim
ROPE_BASE = 10000.0
EPS = float(np.finfo(np.float32).eps)

import os
KPHASE = int(os.environ.get("KPHASE", "3"))
KDEBUG = int(os.environ.get("KDEBUG", "0"))

NK = D // 128       # 8 contraction tiles for qkv projections
SQB = 256           # sq block size in attention
NB = S // SQB       # 8 blocks
NJ = S // 128       # 16 sk tiles
NS5 = S // 512      # 4 n-tiles of 512 in projections


def _consts():
    """Constant tensors baked into the NEFF (same for every core)."""
    i = np.arange(32, dtype=np.float64)
    inv_freq = 1.0 / (ROPE_BASE ** (2.0 * i / HD))
    pos = np.arange(S, dtype=np.float64)
    fr = pos[:, None] * inv_freq[None, :]          # [S, 32]
    cosT = np.cos(fr).T.astype(np.float32)          # [32, S]
    sinT = np.sin(fr).T.astype(np.float32)
    cos4 = np.tile(cosT, (4, 1)).astype(BF16NP)     # [128, S]
    sin4 = np.tile(sinT, (4, 1)).astype(BF16NP)
    nsin4 = (-np.tile(sinT, (4, 1))).astype(BF16NP)

    # causal masks for diagonal sk-tiles: pattern p in {0,1}
    # valid iff c >= 128*p + r   (r: sk row 0..127, c: sq col 0..255)
    r = np.arange(128)[:, None]
    c = np.arange(SQB)[None, :]
    masks = []
    for p in range(2):
        m = (c >= 128 * p + r).astype(BF16NP)       # [128, 256]
        masks.append(np.tile(m, (1, HG)))            # [128, 1024] (4 head blocks)

    bsel4 = np.zeros((4, 128), dtype=BF16NP)        # broadcast f[h] -> rows 32h..32h+32
    for h in range(4):
        bsel4[h, 32 * h:32 * h + 32] = 1.0
    sel4 = bsel4.T.copy()                            # [128, 4] sumsq selector
    ones64 = np.ones((1, 64), dtype=BF16NP)
    ones64col = np.ones((64, 1), dtype=BF16NP)
    id128 = np.eye(128, dtype=BF16NP)
    return cos4, sin4, nsin4, masks, bsel4, sel4, ones64, ones64col, id128


def _build():
    nc = bacc.Bacc("TRN2", debug=False)

    xT_d = nc.dram_tensor("xT", [D, S], BF16, kind="ExternalInput")
    wq_d = nc.dram_tensor("wq", [NK, 128, E], BF16, kind="ExternalInput")
    wkv_d = nc.dram_tensor("wkv", [NK, 128, 128], BF16, kind="ExternalInput")
    wo_d = nc.dram_tensor("wo", [2, 128, D], BF16, kind="ExternalInput")
    qlnb_d = nc.dram_tensor("qlnb", [4, 1], F32, kind="ExternalInput")
    out_d = nc.dram_tensor("out", [S, D], F32, kind="ExternalOutput")
    dbg = {}
    if KDEBUG:
        for nm, shp in [("d_qsb0", [128, S]), ("d_qsb1", [128, S]),
                        ("d_kvsb", [128, S]), ("d_fq", [4, S]), ("d_fbcq", [128, S]),
                        ("d_qstd0", [128, S]), ("d_qstd1", [128, S]),
                        ("d_kdup", [128, S]), ("d_vsb", [128, NJ, 65]),
                        ("d_yn0", [128, S]), ("d_yn1", [128, S]),
                        ("d_pt0", [128, HG * SQB]), ("d_pt1", [128, HG * SQB]),
                        ("d_yt", [128, 4 * 256]), ("d_rbs", [128, 4 * 256])]:
            dbg[nm] = nc.dram_tensor(nm, shp, BF16, kind="ExternalOutput")

    cos4, sin4, nsin4, masks, bsel4, sel4, ones64, ones64col, id128 = _consts()
    cos4_d = nc.inline_tensor(cos4, "cos4")
    sin4_d = nc.inline_tensor(sin4, "sin4")
    nsin4_d = nc.inline_tensor(nsin4, "nsin4")
    mask_d = [nc.inline_tensor(masks[p], f"mask{p}") for p in range(2)]
    bsel4_d = nc.inline_tensor(bsel4, "bsel4")
    sel4_d = nc.inline_tensor(sel4, "sel4")
    ones64_d = nc.inline_tensor(ones64, "ones64")
    ones64col_d = nc.inline_tensor(ones64col, "ones64col")
    id128_d = nc.inline_tensor(id128, "id128")

    with tile.TileContext(nc) as tc, ExitStack() as ctx:
        sp = ctx.enter_context(tc.tile_pool(name="static", bufs=1))

        def stile(shape, dt, tag):
            return sp.tile(shape, dt, name=tag, tag=tag)

        # ---- static SBUF tensors ----
        xt = [stile([128, S], BF16, f"xt{k}") for k in range(NK)]
        wq = stile([128, NK, E], BF16, "wq")
        wkv = stile([128, NK, 128], BF16, "wkv")
        wo = stile([128, 2, D], BF16, "wo")
        cos4_s = stile([128, S], BF16, "cos4")
        sin4_s = stile([128, S], BF16, "sin4")
        nsin4_s = stile([128, S], BF16, "nsin4")
        mask_s = [stile([128, HG * SQB], BF16, f"mask{p}") for p in range(2)]
        bsel4_s = stile([4, 128], BF16, "bsel4")
        sel4_s = stile([128, 4], BF16, "sel4")
        ones64_s = stile([1, 64], BF16, "ones64")
        ones64col_s = stile([64, 1], BF16, "ones64col")
        id128_s = stile([128, 128], BF16, "id128")
        qlnb_s = stile([4, 1], F32, "qlnb")
        epsb = stile([128, 1], F32, "epsb")
        zb = stile([128, 1], F32, "zb")

        qsb = [stile([128, S], BF16, f"qsb{m}") for m in range(2)]   # T/B packed
        kvsb = stile([128, S], BF16, "kvsb")                          # k(0:64) | v(64:128)
        sqq = [stile([128, S], BF16, f"sqq{m}") for m in range(2)]
        sqkv = stile([64, S], BF16, "sqkv")
        fq = stile([4, S], BF16, "fq")
        fk = stile([1, S], BF16, "fk")
        fbcq = stile([128, S], BF16, "fbcq")
        fbck = stile([64, S], BF16, "fbck")
        qr = [stile([128, S], BF16, f"qr{m}") for m in range(2)]      # rotated T/B
        kr = [stile([32, S], BF16, f"kr{m}") for m in range(2)]
        qstd = [stile([128, S], BF16, f"qstd{m}") for m in range(2)]  # per-head layout
        kdup = stile([128, S], BF16, "kdup")
        kb0 = stile([32, S], BF16, "kb0")
        onesq = stile([128, 64], BF16, "onesq")
        vsb = stile([128, NJ, 65], BF16, "vsb")                       # [v | ones]
        yn = [stile([128, S], BF16, f"yn{m}") for m in range(2)]      # normalized y^T

        # ---- load everything ----
        for k in range(NK):
            nc.sync.dma_start(xt[k][:], xT_d[128 * k:128 * (k + 1), :])
            nc.sync.dma_start(wq[:, k, :], wq_d[k])
            nc.sync.dma_start(wkv[:, k, :], wkv_d[k])
        nc.sync.dma_start(wo[:, 0, :], wo_d[0])
        nc.sync.dma_start(wo[:, 1, :], wo_d[1])
        nc.sync.dma_start(cos4_s[:], cos4_d[:])
        nc.sync.dma_start(sin4_s[:], sin4_d[:])
        nc.sync.dma_start(nsin4_s[:], nsin4_d[:])
        for p in range(2):
            nc.sync.dma_start(mask_s[p][:], mask_d[p][:])
        nc.sync.dma_start(bsel4_s[:], bsel4_d[:])
        nc.sync.dma_start(sel4_s[:], sel4_d[:])
        nc.sync.dma_start(ones64_s[:], ones64_d[:])
        nc.sync.dma_start(ones64col_s[:], ones64col_d[:])
        nc.sync.dma_start(id128_s[:], id128_d[:])
        nc.sync.dma_start(qlnb_s[:], qlnb_d[:])
        nc.vector.memset(vsb[:], 1.0)  # ones column at [:, j, 64]; 0:64 overwritten below
        nc.vector.memset(epsb[:], EPS)
        nc.vector.memset(zb[:], 0.0)
        nc.vector.memset(onesq[:], 1.0)

        # ======== phase 1: projections + rms factors + rope ========
        with (
            tc.tile_pool(name="pp", bufs=4, space=bass.MemorySpace.PSUM) as pp,
            tc.tile_pool(name="lns", bufs=2) as lns,
        ):
            # Q projection -> qsb (permuted: tileT = tops of 4 heads, tileB = bottoms)
            for m in range(2):
                pq = [pp.tile([128, 512], F32, name="pq", tag="pq", bufs=4) for _ in range(NS5)]
                for k in range(NK):
                    for n in range(NS5):
                        nc.tensor.matmul(
                            pq[n][:], wq[:, k, 128 * m:128 * (m + 1)],
                            xt[k][:, 512 * n:512 * (n + 1)],
                            start=(k == 0), stop=(k == NK - 1))
                for n in range(NS5):
                    sl = slice(512 * n, 512 * (n + 1))
                    nc.scalar.copy(qsb[m][:, sl], pq[n][:])
                    nc.vector.tensor_mul(sqq[m][:, sl], qsb[m][:, sl], qsb[m][:, sl])
            # KV projection
            pkv = [pp.tile([128, 512], F32, name="pq", tag="pq", bufs=4) for _ in range(NS5)]
            for k in range(NK):
                for n in range(NS5):
                    nc.tensor.matmul(
                        pkv[n][:], wkv[:, k, :], xt[k][:, 512 * n:512 * (n + 1)],
                        start=(k == 0), stop=(k == NK - 1))
            for n in range(NS5):
                sl = slice(512 * n, 512 * (n + 1))
                nc.scalar.copy(kvsb[:, sl], pkv[n][:])
                nc.vector.tensor_mul(sqkv[:, sl], kvsb[0:64, sl], kvsb[0:64, sl])
                # v transpose: [64,128] slices -> [128,64]
                for t in range(4):
                    st = 4 * n + t
                    ptr = pp.tile([128, 64], BF16, name="ptr", tag="ptr", bufs=2)
                    nc.tensor.transpose(
                        ptr[:], kvsb[64:128, 128 * st:128 * (st + 1)],
                        id128_s[64:128, 64:128])
                    nc.vector.tensor_copy(vsb[:, st, 0:64], ptr[:])

            # rms factors: f = exp(-0.5*ln(ssq/HD + eps) + ln(gain/8))
            for n in range(NS5):
                sl = slice(512 * n, 512 * (n + 1))
                psq = pp.tile([4, 512], F32, name="psq", tag="psq", bufs=2)
                nc.tensor.matmul(psq[:], sel4_s[:], sqq[0][:, sl], start=True, stop=False)
                nc.tensor.matmul(psq[:], sel4_s[:], sqq[1][:, sl], start=False, stop=True)
                lnt = lns.tile([4, 512], F32, name="pln", tag="pln")
                nc.scalar.activation(lnt[:], psq[:], AF.Ln, scale=1.0 / HD, bias=epsb[0:4, :])
                nc.scalar.activation(fq[:, sl], lnt[:], AF.Exp, scale=-0.5,
                                     bias=qlnb_s[:, :])
                psk = pp.tile([1, 512], F32, name="psq", tag="psq", bufs=2)
                nc.tensor.matmul(psk[:], ones64col_s[:], sqkv[:, sl], start=True, stop=True)
                lnk = lns.tile([1, 512], F32, name="pln", tag="pln")
                nc.scalar.activation(lnk[:], psk[:], AF.Ln, scale=1.0 / HD, bias=epsb[0:1, :])
                nc.scalar.activation(fk[:, sl], lnk[:], AF.Exp, scale=-0.5, bias=zb[0:1, :])
                # broadcast factors along hd rows via PE
                pb = pp.tile([128, 512], F32, name="pq", tag="pq", bufs=4)
                nc.tensor.matmul(pb[:], bsel4_s[:], fq[:, sl], start=True, stop=True)
                nc.scalar.copy(fbcq[:, sl], pb[:])
                pbk = pp.tile([64, 512], F32, name="pq", tag="pq", bufs=4)
                nc.tensor.matmul(pbk[:], ones64_s[:], fk[:, sl], start=True, stop=True)
                nc.scalar.copy(fbck[:, sl], pbk[:])

            # k bottom half shifted to partition base 0 (DVE ops need aligned bases)
            nc.sync.dma_start(kb0[:], kvsb[32:64, :])

            # rope + scale (DVE, bf16)
            with tc.tile_pool(name="rt", bufs=4) as rt:
                for n in range(NS5):
                    sl = slice(512 * n, 512 * (n + 1))
                    t1 = rt.tile([128, 512], BF16, name="t1", tag="t1")
                    t2 = rt.tile([128, 512], BF16, name="t2", tag="t2")
                    nc.vector.tensor_mul(t1[:], qsb[0][:, sl], cos4_s[:, sl])
                    nc.vector.tensor_mul(t2[:], qsb[1][:, sl], sin4_s[:, sl])
                    nc.vector.tensor_add(t1[:], t1[:], t2[:])
                    nc.vector.tensor_mul(qr[0][:, sl], t1[:], fbcq[:, sl])
                    u1 = rt.tile([128, 512], BF16, name="t1", tag="t1")
                    u2 = rt.tile([128, 512], BF16, name="t2", tag="t2")
                    nc.vector.tensor_mul(u1[:], qsb[0][:, sl], nsin4_s[:, sl])
                    nc.vector.tensor_mul(u2[:], qsb[1][:, sl], cos4_s[:, sl])
                    nc.vector.tensor_add(u1[:], u1[:], u2[:])
                    nc.vector.tensor_mul(qr[1][:, sl], u1[:], fbcq[:, sl])
                    k1 = rt.tile([32, 512], BF16, name="k1", tag="k1")
                    k2 = rt.tile([32, 512], BF16, name="k2", tag="k2")
                    nc.vector.tensor_mul(k1[:], kvsb[0:32, sl], cos4_s[0:32, sl])
                    nc.vector.tensor_mul(k2[:], kb0[:, sl], sin4_s[0:32, sl])
                    nc.vector.tensor_add(k1[:], k1[:], k2[:])
                    nc.vector.tensor_mul(kr[0][:, sl], k1[:], fbck[0:32, sl])
                    k3 = rt.tile([32, 512], BF16, name="k1", tag="k1")
                    k4 = rt.tile([32, 512], BF16, name="k2", tag="k2")
                    nc.vector.tensor_mul(k3[:], kvsb[0:32, sl], nsin4_s[0:32, sl])
                    nc.vector.tensor_mul(k4[:], kb0[:, sl], cos4_s[0:32, sl])
                    nc.vector.tensor_add(k3[:], k3[:], k4[:])
                    nc.vector.tensor_mul(kr[1][:, sl], k3[:], fbck[0:32, sl])

        # reassemble per-head layout (DMA partition moves)
        for h in range(4):
            dst = qstd[h // 2]
            base = 64 * (h % 2)
            nc.sync.dma_start(dst[base:base + 32, :], qr[0][32 * h:32 * h + 32, :])
            nc.sync.dma_start(dst[base + 32:base + 64, :], qr[1][32 * h:32 * h + 32, :])
        nc.sync.dma_start(kdup[0:32, :], kr[0][:])
        nc.sync.dma_start(kdup[32:64, :], kr[1][:])
        nc.sync.dma_start(kdup[64:96, :], kr[0][:])
        nc.sync.dma_start(kdup[96:128, :], kr[1][:])

        # ======== phase 2: attention ========
        if KPHASE >= 2:
            with (
              tc.tile_pool(name="ps", bufs=2, space=bass.MemorySpace.PSUM) as ps,
              tc.tile_pool(name="py", bufs=4, space=bass.MemorySpace.PSUM) as py,
              tc.tile_pool(name="pa", bufs=3) as pa,
          ):
              for b in range(NB):
                  sq = slice(SQB * b, SQB * (b + 1))
                  jmax = 2 * b + 1
                  yt = [py.tile([65, 256], F32, name="yt", tag="yt") for _ in range(4)]
                  # concurrent row-group pairs (h even @rows 0:64, h odd @64:128)
                  # must hit different PSUM banks: head h -> col COLOF[h].
                  COLOF = [0, 512, 256, 768]
                  for j in range(jmax + 1):
                      stile_ = ps.tile([128, HG * SQB], F32, name="st", tag="st")
                      for h in range(4):
                          base = 64 * (h % 2)
                          co = COLOF[h]
                          nc.tensor.matmul(
                              stile_[:, co:co + SQB],
                              kdup[base:base + 64, 128 * j:128 * (j + 1)],
                              qstd[h // 2][base:base + 64, sq],
                              start=True, stop=True, skip_group_check=True)
                      pt = pa.tile([128, HG * SQB], BF16, name="pt", tag="pt")
                      nc.scalar.activation(pt[:], stile_[:], AF.Exp, bias=zb[:, :])
                      if j >= 2 * b:
                          nc.vector.tensor_mul(pt[:], pt[:], mask_s[j - 2 * b][:])
                      if KDEBUG and b == 0:
                          nc.sync.dma_start(dbg[f"d_pt{j}"][:], pt[:])
                      for h in range(4):
                          nc.tensor.matmul(
                              yt[h][:], vsb[:, j, :], pt[:, COLOF[h]:COLOF[h] + SQB],
                              start=(j == 0), stop=(j == jmax))
                  if KDEBUG and b == 0:
                      for h in range(4):
                          ytc = pa.tile([128, 256], BF16, name="ytc", tag="ytc")
                          nc.vector.tensor_copy(ytc[0:65, :], yt[h][:])
                          nc.sync.dma_start(dbg["d_yt"][:, 256 * h:256 * (h + 1)],
                                            ytc[:])
                  # normalize: y / denom, write into yn (per-head rows)
                  for h in range(4):
                      dcb = pa.tile([128, 256], BF16, name="dcb", tag="dcb")
                      nc.vector.tensor_copy(dcb[64:65, :], yt[h][64:65, :])
                      prb = ps.tile([64, 256], F32, name="st", tag="st")
                      nc.tensor.matmul(prb[:], onesq[64:65, :], dcb[64:65, :],
                                       start=True, stop=True)
                      dbs = pa.tile([64, 256], F32, name="dbs", tag="dbs")
                      nc.vector.tensor_copy(dbs[:], prb[:])
                      rbs = pa.tile([64, 256], F32, name="rbs", tag="rbs")
                      nc.vector.reciprocal_approx_fast(rbs[:], dbs[:])
                      if KDEBUG and b == 0:
                          rbc_ = pa.tile([128, 256], BF16, name="rbc_", tag="ytc")
                          nc.vector.tensor_copy(rbc_[0:64, :], rbs[:])
                          nc.sync.dma_start(dbg["d_rbs"][:, 256 * h:256 * (h + 1)],
                                            rbc_[:])
                      if h % 2 == 0:
                          nc.vector.tensor_mul(yn[h // 2][0:64, sq],
                                               yt[h][0:64, :], rbs[:])
                      else:
                          stg = pa.tile([64, 256], BF16, name="stg", tag="stg")
                          nc.vector.tensor_mul(stg[:], yt[h][0:64, :], rbs[:])
                          nc.sync.dma_start(yn[h // 2][64:128, sq], stg[:])

        if KDEBUG:
                nc.sync.dma_start(dbg["d_qsb0"][:], qsb[0][:])
                nc.sync.dma_start(dbg["d_qsb1"][:], qsb[1][:])
                nc.sync.dma_start(dbg["d_kvsb"][:], kvsb[:])
                nc.sync.dma_start(dbg["d_fq"][:], fq[:])
                nc.sync.dma_start(dbg["d_fbcq"][:], fbcq[:])
                nc.sync.dma_start(dbg["d_qstd0"][:], qstd[0][:])
                nc.sync.dma_start(dbg["d_qstd1"][:], qstd[1][:])
                nc.sync.dma_start(dbg["d_kdup"][:], kdup[:])
                nc.sync.dma_start(dbg["d_vsb"][:], vsb[:])
                if KPHASE >= 2:
                    nc.sync.dma_start(dbg["d_yn0"][:], yn[0][:])
                    nc.sync.dma_start(dbg["d_yn1"][:], yn[1][:])

        # ======== phase 3: output projection ========
        if KPHASE >= 3:
            with (
              tc.tile_pool(name="po", bufs=2, space=bass.MemorySpace.PSUM) as po,
              tc.tile_pool(name="ob", bufs=3) as ob,
          ):
              for st in range(16):
                  ssl = slice(128 * st, 128 * (st + 1))
                  pot = po.tile([128, D], F32, name="po", tag="po")
                  for n in range(2):
                      for kk in range(2):
                          nc.tensor.matmul(
                              pot[:, 512 * n:512 * (n + 1)], yn[kk][:, ssl],
                              wo[:, kk, 512 * n:512 * (n + 1)],
                              start=(kk == 0), stop=(kk == 1))
                  ot = ob.tile([128, D], F32, name="ot", tag="ot")
                  nc.vector.tensor_copy(ot[:], pot[:])
                  nc.sync.dma_start(out_d[ssl, :], ot[:])

    nc.finalize()
    return nc


_NC = None


def _get_nc():
    global _NC
    if _NC is None:
        _NC = _build()
    return _NC


def _perm():
    tops = [h * 64 + i for h in range(HG) for i in range(32)]
    bots = [h * 64 + 32 + i for h in range(HG) for i in range(32)]
    return tops + bots


def kernel(x, Wq, Wk, Wv, Wo, q_gain):
    x = np.asarray(x, dtype=np.float32)
    Wq = np.asarray(Wq, dtype=np.float32)
    Wk = np.asarray(Wk, dtype=np.float32)
    Wv = np.asarray(Wv, dtype=np.float32)
    Wo = np.asarray(Wo, dtype=np.float32)
    q_gain = np.asarray(q_gain, dtype=np.float32)

    perm = _perm()
    in_maps = []
    for c in range(8):
        dp, tp = divmod(c, 4)
        xT = np.ascontiguousarray(x[dp].T).astype(BF16NP)
        wq_sel = Wq[tp * E:(tp + 1) * E].T[:, perm]          # [D, 256] permuted
        wq_t = np.ascontiguousarray(wq_sel).astype(BF16NP).reshape(NK, 128, E)
        wk_sel = Wk[tp * HD:(tp + 1) * HD].T                  # [D, 64]
        wv_sel = Wv[tp * HD:(tp + 1) * HD].T
        wkv_t = np.concatenate([wk_sel, wv_sel], axis=1).astype(BF16NP)
        wkv_t = np.ascontiguousarray(wkv_t).reshape(NK, 128, 128)
        wo_sel = Wo[:, tp * E:(tp + 1) * E].T                 # [256, D]
        wo_t = np.ascontiguousarray(wo_sel).astype(BF16NP).reshape(2, 128, D)
        g = q_gain[tp * HG:(tp + 1) * HG].astype(np.float64)
        qlnb = np.log(np.maximum(g, 1e-30) / 8.0).astype(np.float32).reshape(4, 1)
        in_maps.append({
            "xT": xT, "wq": wq_t, "wkv": wkv_t, "wo": wo_t, "qlnb": qlnb,
        })

    nc = _get_nc()
    res = run_bass_kernel_spmd(nc, in_maps, core_ids=list(range(8)))
    out = np.zeros((B, S, D), dtype=np.float32)
    for c in range(8):
        out[c // 4] += res.results[c]["out"]
    return out



# revision 16
# speedup vs baseline: 1.1196x; 1.1196x over previous
"""Trainium2 Bass kernel for causal GQA self-attention (B=2,S=2048,D=1024,H=16,HKV=4,HD=64).

Sharding: 8 cores = DP(2 over batch) x TP(4 over GQA groups).
Each core computes, for one batch element and one GQA group (4 q heads + 1 kv head),
the partial output  y_group @ Wo[:, group_cols].T  (row-sharded Wo).
Host sums the 4 TP partials per batch element.

v2 layout:
  - q4 [64, (head, S)] head-major: scores for all 4 heads = 2 matmuls of N=512 per k-tile
  - PV split into even-pair / odd-pair matmuls; odd pair's y^T lands at partitions 64:128
    (stationary operand vob has v at cols 64:128, denom-ones at col 32)
  - denominator reciprocal broadcast via GpSimd partition_broadcast (PE/ACT stay busy)
  - output projection folded into the attention block loop
"""

import os
import sys
from contextlib import ExitStack

sys.path.insert(0, "/opt/trn_rl_repo")

import numpy as np
import ml_dtypes

import concourse.bass as bass
import concourse.bacc as bacc
import concourse.tile as tile
import concourse.mybir as mybir
from concourse.bass_utils import run_bass_kernel_spmd

BF16 = mybir.dt.bfloat16
F32 = mybir.dt.float32
AF = mybir.ActivationFunctionType
ALU = mybir.AluOpType
BF16NP = ml_dtypes.bfloat16

D, H, HKV, HD, B, S = 1024, 16, 4, 64, 2, 2048
HG = 4              # q heads per core
E = HG * HD         # 256 local q-proj dim
ROPE_BASE = 10000.0
EPS = float(np.finfo(np.float32).eps)

NK = D // 128       # 8 contraction tiles for projections
SQB = 256           # sq block size in attention
NB = S // SQB       # 8 blocks
NJ = S // 128       # 16 sk tiles
HORD = [0, 2, 1, 3]  # head order along q4's head axis (even pair first)

KBCAST = os.environ.get("KBCAST", "gp")   # gp: gpsimd bcast | pe: matmul bcast
KDEBUG = int(os.environ.get("KDEBUG", "0"))


def _consts():
    i = np.arange(32, dtype=np.float64)
    inv_freq = 1.0 / (ROPE_BASE ** (2.0 * i / HD))
    pos = np.arange(S, dtype=np.float64)
    fr = pos[:, None] * inv_freq[None, :]           # [S, 32]
    cosT = np.cos(fr).T.astype(np.float32)           # [32, S]
    sinT = np.sin(fr).T.astype(np.float32)
    cos4 = np.tile(cosT, (4, 1)).astype(BF16NP)      # [128, S]
    sin4 = np.tile(sinT, (4, 1)).astype(BF16NP)

    # causal masks for diagonal sk-tiles: pattern p in {0,1}
    # valid iff c >= 128*p + r   (r: sk row 0..127, c: sq col 0..255)
    r = np.arange(128)[:, None]
    c = np.arange(SQB)[None, :]
    masks = []
    for p in range(2):
        m = (c >= 128 * p + r).astype(BF16NP)        # [128, 256]
        masks.append(np.tile(m, (1, HG)))            # [128, 1024]

    sel4x = np.zeros((128, 33), dtype=BF16NP)        # head sumsq selector (q)
    for h in range(4):
        sel4x[32 * h:32 * h + 32, h] = 1.0
    selk = np.zeros((64, 33), dtype=BF16NP)          # k sumsq -> row 32
    selk[:, 32] = 1.0
    bsel4 = np.zeros((4, 128), dtype=BF16NP)         # f[h] -> rows 32h..32h+32
    for h in range(4):
        bsel4[h, 32 * h:32 * h + 32] = 1.0
    onesk = np.zeros((33, 64), dtype=BF16NP)         # row32 ones (pe-bcast path)
    onesk[32, :] = 1.0
    ones1 = np.zeros((33, 64), dtype=np.float32)     # f32 lhsT for pe bcast
    ones1[0, :] = 1.0
    ones1[32, :] = 1.0
    id128 = np.eye(128, dtype=BF16NP)
    return cos4, sin4, masks, sel4x, selk, bsel4, onesk, ones1, id128


def _build():
    nc = bacc.Bacc("TRN2", debug=False)

    xT_d = nc.dram_tensor("xT", [D, S], BF16, kind="ExternalInput")
    wq_d = nc.dram_tensor("wq", [NK, 128, E], BF16, kind="ExternalInput")
    wkv_d = nc.dram_tensor("wkv", [NK, 128, 128], BF16, kind="ExternalInput")
    wo_d = nc.dram_tensor("wo", [2, 128, D], BF16, kind="ExternalInput")
    qlnb_d = nc.dram_tensor("qlnb", [4, 1], F32, kind="ExternalInput")
    out_d = nc.dram_tensor("out", [S, D], F32, kind="ExternalOutput")

    dbg = {}
    if KDEBUG:
        for nm, shp in [("d_q4", [64, HG, S]), ("d_k4", [64, S]),
                        ("d_qsb0", [128, S]), ("d_qsb1", [128, S]),
                        ("d_kvsb", [128, S]), ("d_veb", [128, NJ, 65]),
                        ("d_vob", [128, NJ, 128]), ("d_yn0", [128, S]),
                        ("d_yn1", [128, S]), ("d_pt", [128, 2, HG * SQB]),
                        ("d_fbcq", [128, S]), ("d_fbck", [64, S])]:
            dbg[nm] = nc.dram_tensor(nm, shp, BF16, kind="ExternalOutput")
        dbg["d_rb"] = nc.dram_tensor("d_rb", [128, 2, 512], F32,
                                     kind="ExternalOutput")
        dbg["d_ds"] = nc.dram_tensor("d_ds", [1, 1024], F32,
                                     kind="ExternalOutput")
        dbg["d_ytO"] = nc.dram_tensor("d_ytO", [128, 512], F32,
                                      kind="ExternalOutput")

    cos4, sin4, masks, sel4x, selk, bsel4, onesk, ones1, id128 = _consts()
    cos4_d = nc.inline_tensor(cos4, "cos4")
    sin4_d = nc.inline_tensor(sin4, "sin4")
    mask_d = [nc.inline_tensor(masks[p], f"mask{p}") for p in range(2)]
    sel4x_d = nc.inline_tensor(sel4x, "sel4x")
    selk_d = nc.inline_tensor(selk, "selk")
    bsel4_d = nc.inline_tensor(bsel4, "bsel4")
    id128_d = nc.inline_tensor(id128, "id128")

    with tile.TileContext(nc) as tc, ExitStack() as ctx:
        sp = ctx.enter_context(tc.tile_pool(name="static", bufs=1))

        def stile(shape, dt, tag):
            return sp.tile(shape, dt, name=tag, tag=tag)

        # ---- static SBUF tensors ----
        xt = [stile([128, S], BF16, f"xt{k}") for k in range(NK)]
        wq = stile([128, NK, E], BF16, "wq")
        wkv = stile([128, NK, 128], BF16, "wkv")
        wo = stile([128, 2, D], BF16, "wo")
        cos4_s = stile([128, S], BF16, "cos4")
        sin4_s = stile([128, S], BF16, "sin4")
        mask_s = [stile([128, HG * SQB], BF16, f"mask{p}") for p in range(2)]
        sel4x_s = stile([128, 33], BF16, "sel4x")
        selk_s = stile([64, 33], BF16, "selk")
        bsel4_s = stile([4, 128], BF16, "bsel4")
        id128_s = stile([128, 128], BF16, "id128")
        epsb = stile([33, 1], F32, "epsb")
        qlnb33 = stile([33, 1], F32, "qlnb33")

        qsb = [stile([128, S], BF16, f"qsb{m}") for m in range(2)]   # packed T/B
        kvsb = stile([128, S], BF16, "kvsb")                         # k(0:64) | v(64:128)
        qr = [stile([128, S], BF16, f"qr{m}") for m in range(2)]     # rotated T/B
        q4 = stile([64, HG, S], BF16, "q4")                          # head-major q
        k4 = stile([64, S], BF16, "k4")                              # rotated k
        veb = stile([128, NJ, 65], BF16, "veb")                      # [v | ones]
        vob = stile([128, NJ, 128], BF16, "vob")                     # [0|1@32|0|v]
        yn = [stile([128, S], BF16, f"yn{m}") for m in range(2)]     # normalized y^T

        # ---- load everything ----
        for k in range(NK):
            nc.sync.dma_start(xt[k][:], xT_d[128 * k:128 * (k + 1), :])
            nc.sync.dma_start(wq[:, k, :], wq_d[k])
            nc.sync.dma_start(wkv[:, k, :], wkv_d[k])
        nc.sync.dma_start(wo[:, 0, :], wo_d[0])
        nc.sync.dma_start(wo[:, 1, :], wo_d[1])
        nc.sync.dma_start(cos4_s[:], cos4_d[:])
        nc.sync.dma_start(sin4_s[:], sin4_d[:])
        for p in range(2):
            nc.sync.dma_start(mask_s[p][:], mask_d[p][:])
        nc.sync.dma_start(sel4x_s[:], sel4x_d[:])
        nc.sync.dma_start(selk_s[:], selk_d[:])
        nc.sync.dma_start(bsel4_s[:], bsel4_d[:])
        nc.sync.dma_start(id128_s[:], id128_d[:])
        nc.vector.memset(epsb[:], EPS)
        nc.vector.memset(qlnb33[:], 0.0)
        nc.sync.dma_start(qlnb33[0:4, :], qlnb_d[:])
        nc.vector.memset(veb[:], 1.0)     # ones col at [:, j, 64]; v overwrites 0:64
        nc.vector.memset(vob[:], 0.0)
        nc.vector.memset(vob[:, :, 32:33], 1.0)

        if KBCAST == "pe":
            ones1_d = nc.inline_tensor(ones1, "ones1")
            ones1_s = stile([33, 64], F32, "ones1")
            nc.sync.dma_start(ones1_s[:], ones1_d[:])

        # ======== phase 1: projections + rms factors + rope ========
        with (
            tc.tile_pool(name="pp", bufs=6, space=bass.MemorySpace.PSUM) as pp,
            tc.tile_pool(name="misc", bufs=2, space=bass.MemorySpace.PSUM) as mp,
            tc.tile_pool(name="sq", bufs=2) as sqp,
            tc.tile_pool(name="lns", bufs=2) as lns,
            tc.tile_pool(name="rp", bufs=2) as rp,
        ):
            for ch in range(2):           # column halves of 1024
                # --- projections (k outer for LDW reuse) ---
                pq = [[pp.tile([128, 512], F32, name="pq", tag="pq", bufs=6)
                       for _ in range(2)] for _ in range(3)]
                for k in range(NK):
                    for m in range(3):
                        lhsT = (wq[:, k, 0:128] if m == 0 else
                                wq[:, k, 128:256] if m == 1 else wkv[:, k, :])
                        for n2 in range(2):
                            c0 = 1024 * ch + 512 * n2
                            nc.tensor.matmul(
                                pq[m][n2][:], lhsT, xt[k][:, c0:c0 + 512],
                                start=(k == 0), stop=(k == NK - 1))
                # --- per n-tile epilogue ---
                for n2 in range(2):
                    n = 2 * ch + n2
                    sl = slice(512 * n, 512 * (n + 1))
                    # copies PSUM -> SBUF bf16
                    nc.scalar.copy(qsb[0][:, sl], pq[0][n2][:])
                    nc.scalar.copy(qsb[1][:, sl], pq[1][n2][:])
                    nc.scalar.copy(kvsb[:, sl], pq[2][n2][:])
                    # squares (bf16 DVE)
                    sq0 = sqp.tile([128, 512], BF16, name="sq0", tag="sq0")
                    sq1 = sqp.tile([128, 512], BF16, name="sq1", tag="sq1")
                    sqk = sqp.tile([64, 512], BF16, name="sqk", tag="sqk")
                    nc.vector.tensor_mul(sq0[:], qsb[0][:, sl], qsb[0][:, sl])
                    nc.vector.tensor_mul(sq1[:], qsb[1][:, sl], qsb[1][:, sl])
                    nc.vector.tensor_mul(sqk[:], kvsb[0:64, sl], kvsb[0:64, sl])
                    # per-head sumsq: rows 0:4 = q heads, row 32 = k
                    fpt = mp.tile([33, 512], F32, name="misc", tag="misc")
                    nc.tensor.matmul(fpt[:], sel4x_s[:], sq0[:],
                                     start=True, stop=False)
                    nc.tensor.matmul(fpt[:], sel4x_s[:], sq1[:],
                                     start=False, stop=False)
                    nc.tensor.matmul(fpt[:], selk_s[:], sqk[:],
                                     start=False, stop=True)
                    # f = exp(-0.5*ln(ssq/HD + eps) + ln(gain/8))  (row32: gain-term 0)
                    lnt = lns.tile([33, 512], F32, name="lnt", tag="lnt")
                    nc.scalar.activation(lnt[:], fpt[:], AF.Ln,
                                         scale=1.0 / HD, bias=epsb[:, :])
                    fsb = lns.tile([33, 512], BF16, name="fsb", tag="fsb")
                    nc.scalar.activation(fsb[:], lnt[:], AF.Exp, scale=-0.5,
                                         bias=qlnb33[:, :])
                    # broadcast q factors to 128 rows via PE
                    pb = mp.tile([128, 512], F32, name="misc", tag="misc")
                    nc.tensor.matmul(pb[:], bsel4_s[:], fsb[0:4, :],
                                     start=True, stop=True)
                    fbcq = rp.tile([128, 512], BF16, name="fbcq", tag="fbcq")
                    nc.scalar.copy(fbcq[:], pb[:])
                    # broadcast k factor to 64 rows via GpSimd (src must be
                    # partition 0 on HW -> stage via DVE copy)
                    fk0 = rp.tile([1, 512], BF16, name="fk0", tag="fk0")
                    nc.vector.tensor_copy(fk0[:], fsb[32:33, :])
                    fbck = rp.tile([64, 512], BF16, name="fbck", tag="fbck")
                    nc.gpsimd.partition_broadcast(fbck[:], fk0[0:1, :],
                                                  channels=64)
                    if KDEBUG:
                        nc.sync.dma_start(dbg["d_fbcq"][:, sl], fbcq[:])
                        nc.sync.dma_start(dbg["d_fbck"][:, sl], fbck[:])
                    # rope q (packed layout), f pre-multiplied
                    q0f = rp.tile([128, 512], BF16, name="q0f", tag="q0f")
                    q1f = rp.tile([128, 512], BF16, name="q1f", tag="q1f")
                    nc.vector.tensor_mul(q0f[:], qsb[0][:, sl], fbcq[:])
                    nc.vector.tensor_mul(q1f[:], qsb[1][:, sl], fbcq[:])
                    t0 = rp.tile([128, 512], BF16, name="t0", tag="t0")
                    t1 = rp.tile([128, 512], BF16, name="t1", tag="t1")
                    nc.vector.tensor_mul(t0[:], q0f[:], cos4_s[:, sl])
                    nc.vector.tensor_mul(t1[:], q1f[:], sin4_s[:, sl])
                    nc.vector.tensor_add(qr[0][:, sl], t0[:], t1[:])
                    u0 = rp.tile([128, 512], BF16, name="u0", tag="u0")
                    u1 = rp.tile([128, 512], BF16, name="u1", tag="u1")
                    nc.vector.scalar_tensor_tensor(
                        u0[:], q0f[:], -1.0, sin4_s[:, sl], ALU.mult, ALU.mult)
                    nc.vector.tensor_mul(u1[:], q1f[:], cos4_s[:, sl])
                    nc.vector.tensor_add(qr[1][:, sl], u0[:], u1[:])
                    # rope k -> k4 directly (rows 0:32 top, 32:64 bottom)
                    kf = rp.tile([64, 512], BF16, name="kf", tag="kf")
                    nc.vector.tensor_mul(kf[:], kvsb[0:64, sl], fbck[:])
                    ka = rp.tile([32, 512], BF16, name="ka", tag="ka")
                    kb = rp.tile([32, 512], BF16, name="kb", tag="kb")
                    nc.vector.tensor_mul(ka[:], kf[0:32, :], cos4_s[0:32, sl])
                    nc.vector.tensor_mul(kb[:], kf[32:64, :], sin4_s[32:64, sl])
                    nc.vector.tensor_add(k4[0:32, sl], ka[:], kb[:])
                    kc = rp.tile([32, 512], BF16, name="kc", tag="kc")
                    kd = rp.tile([32, 512], BF16, name="kd", tag="kd")
                    nc.vector.scalar_tensor_tensor(
                        kc[:], kf[0:32, :], -1.0, sin4_s[0:32, sl],
                        ALU.mult, ALU.mult)
                    nc.vector.tensor_mul(kd[:], kf[32:64, :], cos4_s[32:64, sl])
                    nc.vector.tensor_add(k4[32:64, sl], kc[:], kd[:])
                    # v transpose into veb / vob
                    for t in range(4):
                        st = 4 * n + t
                        ptr = mp.tile([128, 64], BF16, name="misc", tag="misc")
                        nc.tensor.transpose(
                            ptr[:], kvsb[64:128, 128 * st:128 * (st + 1)],
                            id128_s[64:128, 64:128])
                        nc.vector.tensor_copy(veb[:, st, 0:64], ptr[:])
                        nc.vector.tensor_copy(vob[:, st, 64:128], ptr[:])
                # --- q reshuffle to head-major (per half) ---
                chs = slice(1024 * ch, 1024 * (ch + 1))
                for s4 in range(4):
                    nc.sync.dma_start(q4[0:32, s4, chs],
                                      qr[0][32 * s4:32 * s4 + 32, chs])
                    nc.sync.dma_start(q4[32:64, s4, chs],
                                      qr[1][32 * s4:32 * s4 + 32, chs])

        if KDEBUG:
            nc.sync.dma_start(dbg["d_q4"][:], q4[:])
            nc.sync.dma_start(dbg["d_k4"][:], k4[:])
            nc.sync.dma_start(dbg["d_qsb0"][:], qsb[0][:])
            nc.sync.dma_start(dbg["d_qsb1"][:], qsb[1][:])
            nc.sync.dma_start(dbg["d_kvsb"][:], kvsb[:])
            nc.sync.dma_start(dbg["d_veb"][:], veb[:])
            nc.sync.dma_start(dbg["d_vob"][:], vob[:])

        # ======== phase 2: attention + output projection ========
        with (
            tc.tile_pool(name="ps", bufs=2, space=bass.MemorySpace.PSUM) as ps,
            tc.tile_pool(name="pyE", bufs=1, space=bass.MemorySpace.PSUM) as pyE,
            tc.tile_pool(name="pyO", bufs=1, space=bass.MemorySpace.PSUM) as pyO,
            tc.tile_pool(name="pow", bufs=2, space=bass.MemorySpace.PSUM) as pow_,
            tc.tile_pool(name="pa", bufs=3) as pa,
            tc.tile_pool(name="rd", bufs=2) as rd,
            tc.tile_pool(name="ob", bufs=3) as ob,
        ):
            for b in range(NB):
                sq = slice(SQB * b, SQB * (b + 1))
                jmax = 2 * b + 1
                ytE = pyE.tile([65, 512], F32, name="ytE", tag="ytE")
                ytO = pyO.tile([128, 512], F32, name="ytO", tag="ytO")
                for j in range(jmax + 1):
                    stl = ps.tile([128, HG * SQB], F32, name="st", tag="st")
                    kT = k4[:, 128 * j:128 * (j + 1)]
                    nc.tensor.matmul(stl[:, 0:512], kT, q4[:, 0:2, sq],
                                     start=True, stop=True)
                    nc.tensor.matmul(stl[:, 512:1024], kT, q4[:, 2:4, sq],
                                     start=True, stop=True)
                    pt = pa.tile([128, HG * SQB], BF16, name="pt", tag="pt")
                    nc.scalar.activation(pt[:], stl[:], AF.Exp)
                    if j >= 2 * b:
                        nc.vector.tensor_mul(pt[:], pt[:], mask_s[j - 2 * b][:])
                    if KDEBUG and b == 0:
                        nc.sync.dma_start(dbg["d_pt"][:, j, :], pt[:])
                    nc.tensor.matmul(ytE[:], veb[:, j, :], pt[:, 0:512],
                                     start=(j == 0), stop=(j == jmax))
                    nc.tensor.matmul(ytO[:], vob[:, j, :], pt[:, 512:1024],
                                     start=(j == 0), stop=(j == jmax))
                # tail: normalize into yn (stage denom rows at partition 0 --
                # custom-DVE recip misreads nonzero partition bases on HW)
                dsE = rd.tile([1, 512], F32, name="dsE", tag="dsE")
                dsO = rd.tile([1, 512], F32, name="dsO", tag="dsO")
                nc.vector.tensor_copy(dsE[:], ytE[64:65, :])
                nc.vector.tensor_copy(dsO[:], ytO[32:33, :])
                rdE = rd.tile([1, 512], F32, name="rdE", tag="rdE")
                rdO = rd.tile([1, 512], F32, name="rdO", tag="rdO")
                nc.vector.reciprocal_approx_fast(rdE[:], dsE[:])
                nc.vector.reciprocal_approx_fast(rdO[:], dsO[:])
                rb = rd.tile([128, 512], F32, name="rb", tag="rb")
                if KBCAST == "gp":
                    # dst partition base must be 0 on HW: fill all 128 rows
                    # with the odd recip first, then overwrite rows 0:64
                    nc.gpsimd.partition_broadcast(rb[:], rdO[0:1, :],
                                                  channels=128)
                    nc.gpsimd.partition_broadcast(rb[0:64, :], rdE[0:1, :],
                                                  channels=64)
                else:
                    pbb = pow_.tile([128, 512], F32, name="po", tag="po")
                    nc.tensor.matmul(pbb[0:64, :], ones1_s[0:1, :],
                                     rdE[0:1, :], start=True, stop=True,
                                     skip_group_check=True)
                    nc.tensor.matmul(pbb[64:128, :], ones1_s[0:1, :],
                                     rdO[0:1, :], start=True, stop=True,
                                     skip_group_check=True)
                    nc.vector.tensor_copy(rb[:], pbb[:])
                if KDEBUG and b == 0:
                    nc.sync.dma_start(dbg["d_rb"][:, 0, :], rb[:])
                    nc.sync.dma_start(dbg["d_ds"][:, 0:512], dsE[:])
                    nc.sync.dma_start(dbg["d_ds"][:, 512:1024], dsO[:])
                    ytOc = ob.tile([128, 512], F32, name="ytOc", tag="ot")
                    nc.vector.tensor_copy(ytOc[:], ytO[:])
                    nc.sync.dma_start(dbg["d_ytO"][:], ytOc[:])
                nc.vector.tensor_mul(yn[0][0:64, sq], ytE[0:64, 0:256],
                                     rb[0:64, 0:256])
                nc.vector.tensor_mul(yn[1][0:64, sq], ytE[0:64, 256:512],
                                     rb[0:64, 256:512])
                nc.vector.tensor_mul(yn[0][64:128, sq], ytO[64:128, 0:256],
                                     rb[64:128, 0:256])
                nc.vector.tensor_mul(yn[1][64:128, sq], ytO[64:128, 256:512],
                                     rb[64:128, 256:512])
                # output projection for this block
                for ss in range(2):
                    rows = slice(SQB * b + 128 * ss, SQB * b + 128 * (ss + 1))
                    for dh in range(2):
                        dsl = slice(512 * dh, 512 * (dh + 1))
                        po = pow_.tile([128, 512], F32, name="po", tag="po")
                        nc.tensor.matmul(po[:], yn[0][:, rows], wo[:, 0, dsl],
                                         start=True, stop=False)
                        nc.tensor.matmul(po[:], yn[1][:, rows], wo[:, 1, dsl],
                                         start=False, stop=True)
                        ot = ob.tile([128, 512], F32, name="ot", tag="ot")
                        nc.vector.tensor_copy(ot[:], po[:])
                        nc.sync.dma_start(out_d[rows, dsl], ot[:])
            if KDEBUG:
                nc.sync.dma_start(dbg["d_yn0"][:], yn[0][:])
                nc.sync.dma_start(dbg["d_yn1"][:], yn[1][:])

    nc.finalize()
    return nc


_NC = None


def _get_nc():
    global _NC
    if _NC is None:
        _NC = _build()
    return _NC


def _perm():
    tops = [HORD[s] * 64 + i for s in range(HG) for i in range(32)]
    bots = [HORD[s] * 64 + 32 + i for s in range(HG) for i in range(32)]
    return tops + bots


def build_inmaps(inputs):
    x = np.asarray(inputs["x"], dtype=np.float32)
    Wq = np.asarray(inputs["Wq"], dtype=np.float32)
    Wk = np.asarray(inputs["Wk"], dtype=np.float32)
    Wv = np.asarray(inputs["Wv"], dtype=np.float32)
    Wo = np.asarray(inputs["Wo"], dtype=np.float32)
    q_gain = np.asarray(inputs["q_gain"], dtype=np.float32)

    perm = _perm()
    in_maps = []
    for c in range(8):
        dp, tp = divmod(c, 4)
        xT = np.ascontiguousarray(x[dp].T).astype(BF16NP)
        wq_sel = Wq[tp * E:(tp + 1) * E].T[:, perm]          # [D, 256] permuted
        wq_t = np.ascontiguousarray(wq_sel).astype(BF16NP).reshape(NK, 128, E)
        wk_sel = Wk[tp * HD:(tp + 1) * HD].T                  # [D, 64]
        wv_sel = Wv[tp * HD:(tp + 1) * HD].T
        wkv_t = np.concatenate([wk_sel, wv_sel], axis=1).astype(BF16NP)
        wkv_t = np.ascontiguousarray(wkv_t).reshape(NK, 128, 128)
        wo_sel = Wo[:, tp * E:(tp + 1) * E].T                 # [256, D]
        wo_t = np.ascontiguousarray(wo_sel).astype(BF16NP).reshape(2, 128, D)
        g = q_gain[tp * HG:(tp + 1) * HG].astype(np.float64)[HORD]
        qlnb = np.log(np.maximum(g, 1e-30) / 8.0).astype(np.float32).reshape(4, 1)
        in_maps.append({
            "xT": xT, "wq": wq_t, "wkv": wkv_t, "wo": wo_t, "qlnb": qlnb,
        })
    return in_maps


def kernel(x, Wq, Wk, Wv, Wo, q_gain):
    in_maps = build_inmaps({"x": x, "Wq": Wq, "Wk": Wk, "Wv": Wv, "Wo": Wo,
                            "q_gain": q_gain})
    nc = _get_nc()
    res = run_bass_kernel_spmd(nc, in_maps, core_ids=list(range(8)))
    out = np.zeros((B, S, D), dtype=np.float32)
    for c in range(8):
        out[c // 4] += res.results[c]["out"]
    return out
